# revision 1
# baseline (speedup 1.0000x reference)
"""Causal multi-head attention (batch=4, seq=2048, d_model=768, 12 heads of 64)
on 8 TRN2 NeuronCores.

Sharding: core c handles batch c//2 and heads (c%2)*6 .. (c%2)*6+6
(data parallel over batch x tensor parallel over head halves).
Each core computes a partial output (its 6 heads' contribution, [2048, 768]);
the host sums the two half-partials per batch and adds biases.

Device math (per core), all matmuls in fp16 with fp32 PSUM accumulation:
  QT[hd, q] = (Wq/8).T @ x.T + bq/8        (scale folded into Wq host-side)
  KT[hd, q] = Wk.T @ x.T + bk
  V[k, hd]  = x @ Wv                        ([V|1] augmented per head)
  ST[k, q]  = KT.T @ QT  (per head, causal strips k0..2048)
  PT = exp(ST) * tri-mask on diagonal blocks  (softmax w/o max-shift: scores
       are O(1) by construction, exp cannot overflow)
  z_aug = [V|1].T @ PT  accumulated over k-strips -> rows 0..63 = z, row 64 = sum
  zT = z_aug[0:64] / z_aug[64]              (recip + partition broadcast)
  out_partial[q, m] = sum_h zT_h.T @ Wo_h
Host: out[b] = partial[2b] + partial[2b+1] + b_O + sum_h b_V[h] @ W_O[h]
(b_V folds exactly through softmax rows summing to 1.)
"""
import numpy as np

import concourse.bass as bass
import concourse.mybir as mybir
import concourse.tile as tile
from concourse import bacc
from concourse.bass_utils import run_bass_kernel_spmd
from concourse.masks import make_upper_triangular

BATCH, SEQ, DM, NH, DH = 4, 2048, 768, 12, 64
H = 6                 # heads per core
HD = H * DH           # 384
MC = DM // 128        # 6 m-chunks
NKT = SEQ // 128      # 16 k-tiles
NQC = SEQ // 512      # 4 q-chunks
F16 = mybir.dt.float16
F32 = mybir.dt.float32

_BUILD_CACHE = {}


def build(reps: int = 1):
    if reps in _BUILD_CACHE:
        return _BUILD_CACHE[reps]
    nc = bacc.Bacc("TRN2", target_bir_lowering=False, debug=False)
    xt_d = nc.dram_tensor("xt", [DM, SEQ], F32, kind="ExternalInput")
    wq_d = nc.dram_tensor("wq", [DM, HD], F32, kind="ExternalInput")
    wk_d = nc.dram_tensor("wk", [DM, HD], F32, kind="ExternalInput")
    wv_d = nc.dram_tensor("wv", [DM, HD], F32, kind="ExternalInput")
    wo_d = nc.dram_tensor("wo", [HD, DM], F32, kind="ExternalInput")
    bq_d = nc.dram_tensor("bq", [HD], F32, kind="ExternalInput")
    bk_d = nc.dram_tensor("bk", [HD], F32, kind="ExternalInput")
    o_d = nc.dram_tensor("out", [SEQ, DM], F32, kind="ExternalOutput")

    with tile.TileContext(nc) as tc:
        def body(_iv=None):
            import contextlib
            with contextlib.ExitStack() as ctx:
                consts = ctx.enter_context(tc.tile_pool(name="consts", bufs=1))
                stage = ctx.enter_context(tc.tile_pool(name="stage", bufs=2))
                persist = ctx.enter_context(tc.tile_pool(name="persist", bufs=1))

                # ---- load + cast inputs ----
                xt16 = []
                for c in range(MC):
                    xs = stage.tile([128, SEQ], F32, name=f"xs{c}", tag="xs")
                    nc.sync.dma_start(out=xs, in_=xt_d.ap()[c * 128:(c + 1) * 128, :])
                    xc = persist.tile([128, SEQ], F16, name=f"xt16_{c}")
                    nc.vector.tensor_copy(xc, xs)
                    xt16.append(xc)

                w16 = {}
                for name, d in [("wq", wq_d), ("wk", wk_d), ("wv", wv_d)]:
                    wf = stage.tile([128, MC, HD], F32, name=f"{name}f", tag="wf")
                    nc.sync.dma_start(
                        out=wf, in_=d.ap().rearrange("(c p) h -> p c h", p=128))
                    wt = persist.tile([128, MC, HD], F16, name=f"{name}16")
                    nc.vector.tensor_copy(wt, wf)
                    w16[name] = wt
                wo16 = []
                for h in range(H):
                    wof = stage.tile([64, DM], F32, name=f"wof{h}", tag="wof")
                    nc.sync.dma_start(out=wof, in_=wo_d.ap()[h * 64:(h + 1) * 64, :])
                    wot = persist.tile([64, DM], F16, name=f"wo16_{h}")
                    nc.vector.tensor_copy(wot, wof)
                    wo16.append(wot)

                bq_s = consts.tile([128, HD // 128], F32)
                nc.sync.dma_start(
                    out=bq_s, in_=bq_d.ap().rearrange("(c p) -> p c", p=128))
                bk_s = consts.tile([128, HD // 128], F32)
                nc.sync.dma_start(
                    out=bk_s, in_=bk_d.ap().rearrange("(c p) -> p c", p=128))

                tri = consts.tile([128, 128], F16)
                make_upper_triangular(nc, tri, val=1.0, diag=True)

                # ---- projections ----
                qt = [persist.tile([128, SEQ], F16, name=f"qt{j}") for j in range(3)]
                kt_ = [persist.tile([128, SEQ], F16, name=f"kt{j}") for j in range(3)]
                vt = [persist.tile([128, H, DH + 1], F16, name=f"v{i}")
                      for i in range(NKT)]

                with tc.tile_pool(name="proj_ps", bufs=4, space="PSUM") as proj_ps:
                    for dst, w, b_s in [(qt, "wq", bq_s), (kt_, "wk", bk_s)]:
                        for j in range(3):
                            for qc in range(NQC):
                                ps = proj_ps.tile([128, 512], F32, name="ps",
                                                  tag="ps")
                                for c in range(MC):
                                    nc.tensor.matmul(
                                        ps,
                                        w16[w][:, c, j * 128:(j + 1) * 128],
                                        xt16[c][:, qc * 512:(qc + 1) * 512],
                                        start=(c == 0), stop=(c == MC - 1))
                                nc.vector.tensor_scalar(
                                    out=dst[j][:, qc * 512:(qc + 1) * 512],
                                    in0=ps, scalar1=b_s[:, j:j + 1], scalar2=None,
                                    op0=mybir.AluOpType.add)
                    for ktile in range(NKT):
                        ps = proj_ps.tile([128, HD], F32, name="psv", tag="ps")
                        for c in range(MC):
                            nc.tensor.matmul(
                                ps,
                                xt16[c][:, ktile * 128:(ktile + 1) * 128],
                                w16["wv"][:, c, :],
                                start=(c == 0), stop=(c == MC - 1))
                        nc.vector.tensor_copy(
                            vt[ktile][:, :, 0:DH],
                            ps.rearrange("p (h d) -> p h d", h=H))
                        nc.vector.memset(vt[ktile][:, :, DH:DH + 1], 1.0)

                # ---- attention per head ----
                zt = [persist.tile([64, SEQ], F16, name=f"zt{h}") for h in range(H)]
                with tc.tile_pool(name="s_ps", bufs=3, space="PSUM") as s_ps, \
                     tc.tile_pool(name="z_ps", bufs=4, space="PSUM") as z_ps, \
                     tc.tile_pool(name="pt_pool", bufs=10) as pt_pool, \
                     tc.tile_pool(name="r_pool", bufs=3) as r_pool, \
                     tc.tile_pool(name="rb_pool", bufs=3) as rb_pool:

                    def qc_range(ktile):
                        return range(ktile // 4, NQC)

                    for h in range(H):
                        j, hp = h // 2, (h % 2) * 64
                        z_aug = [z_ps.tile([65, 512], F32, name=f"z{qc}", tag="z")
                                 for qc in range(NQC)]
                        p_prev = {}

                        def emit_s(ktile):
                            k0 = ktile * 128
                            out = {}
                            for qc in qc_range(ktile):
                                cs = max(qc * 512, k0)
                                w = (qc + 1) * 512 - cs
                                s_t = s_ps.tile([128, 512], F32, name="s_t",
                                                tag="s")
                                nc.tensor.matmul(
                                    s_t[:, 0:w],
                                    kt_[j][hp:hp + 64, k0:k0 + 128],
                                    qt[j][hp:hp + 64, cs:(qc + 1) * 512],
                                    start=True, stop=True)
                                p_t = pt_pool.tile([128, 512], F16, name="p_t",
                                                   tag="pt")
                                nc.scalar.activation(
                                    p_t[:, 0:w], s_t[:, 0:w],
                                    mybir.ActivationFunctionType.Exp)
                                if qc == ktile // 4:
                                    nc.vector.tensor_tensor(
                                        out=p_t[:, 0:128], in0=p_t[:, 0:128],
                                        in1=tri, op=mybir.AluOpType.mult)
                                out[qc] = (p_t, w)
                            return out

                        def emit_pv(ktile):
                            for qc in qc_range(ktile):
                                p_t, w = p_prev[ktile][qc]
                                off = 512 - w
                                nc.tensor.matmul(
                                    z_aug[qc][:, off:512],
                                    vt[ktile][:, h, :],
                                    p_t[:, 0:w],
                                    start=(ktile == 0),
                                    stop=(ktile == 4 * qc + 3))

                        for ktile in range(NKT):
                            p_prev[ktile] = emit_s(ktile)
                            if ktile > 0:
                                emit_pv(ktile - 1)
                                del p_prev[ktile - 1]
                        emit_pv(NKT - 1)

                        for qc in range(NQC):
                            r_t = r_pool.tile([65, 512], F32, name="r_t", tag="r")
                            nc.vector.reciprocal(out=r_t[64:65, :],
                                                 in_=z_aug[qc][64:65, :])
                            r0_t = r_pool.tile([1, 512], F32, name="r0", tag="r0")
                            nc.sync.dma_start(out=r0_t, in_=r_t[64:65, :])
                            rb_t = rb_pool.tile([64, 512], F32, name="rb", tag="rb")
                            nc.gpsimd.partition_broadcast(rb_t, r0_t)
                            nc.vector.tensor_tensor(
                                out=zt[h][:, qc * 512:(qc + 1) * 512],
                                in0=z_aug[qc][0:64, :], in1=rb_t,
                                op=mybir.AluOpType.mult)

                # ---- output projection ----
                with tc.tile_pool(name="o_ps", bufs=4, space="PSUM") as o_ps, \
                     tc.tile_pool(name="o_sb", bufs=3) as o_sb:
                    for qtile in range(SEQ // 128):
                        o_s = o_sb.tile([128, DM], F32, name="o_s", tag="os")
                        for n0, w in [(0, 512), (512, 256)]:
                            o_t = o_ps.tile([128, 512], F32, name="o_t", tag="o")
                            for h in range(H):
                                nc.tensor.matmul(
                                    o_t[:, 0:w],
                                    zt[h][:, qtile * 128:(qtile + 1) * 128],
                                    wo16[h][:, n0:n0 + w],
                                    start=(h == 0), stop=(h == H - 1))
                            nc.vector.tensor_copy(o_s[:, n0:n0 + w], o_t[:, 0:w])
                        nc.sync.dma_start(
                            out=o_d.ap()[qtile * 128:(qtile + 1) * 128, :], in_=o_s)

        if reps == 1:
            body()
        else:
            with tc.For_i(0, reps, 1) as _iv:
                body(_iv)

    nc.compile()
    _BUILD_CACHE[reps] = nc
    return nc


def make_in_maps(normalized_resid_pre, W_Q, W_K, W_V, W_O, b_Q, b_K, b_V, b_O):
    scale = np.float32(1.0 / np.sqrt(DH))
    in_maps = []
    for core in range(8):
        b, h0 = core // 2, (core % 2) * H
        hs = slice(h0, h0 + H)
        in_maps.append({
            "xt": np.ascontiguousarray(normalized_resid_pre[b].T).astype(
                np.float32),
            "wq": (np.ascontiguousarray(
                W_Q[hs].transpose(1, 0, 2).reshape(DM, HD)) * scale).astype(
                np.float32),
            "wk": np.ascontiguousarray(
                W_K[hs].transpose(1, 0, 2).reshape(DM, HD)).astype(np.float32),
            "wv": np.ascontiguousarray(
                W_V[hs].transpose(1, 0, 2).reshape(DM, HD)).astype(np.float32),
            "wo": np.ascontiguousarray(W_O[hs].reshape(HD, DM)).astype(np.float32),
            "bq": (b_Q[hs].reshape(HD) * scale).astype(np.float32),
            "bk": b_K[hs].reshape(HD).astype(np.float32),
        })
    return in_maps


def assemble(results, b_V, b_O, W_O):
    bv_wo = np.einsum("hd,hdm->m", b_V.astype(np.float64),
                      W_O.astype(np.float64)).astype(np.float32)
    out = np.empty((BATCH, SEQ, DM), dtype=np.float32)
    for b in range(BATCH):
        out[b] = (results[2 * b]["out"] + results[2 * b + 1]["out"]
                  + b_O + bv_wo)
    return out


def kernel(normalized_resid_pre, W_Q, W_K, W_V, W_O, b_Q, b_K, b_V, b_O):
    nc = build(reps=1)
    in_maps = make_in_maps(normalized_resid_pre, W_Q, W_K, W_V, W_O,
                           b_Q, b_K, b_V, b_O)
    res = run_bass_kernel_spmd(nc, in_maps, core_ids=list(range(8)))
    return assemble(res.results, b_V, b_O, W_O)


# revision 15
# speedup vs baseline: 12.2564x; 12.2564x over previous
"""Causal multi-head attention (batch=4, seq=2048, d_model=768, 12 heads of 64)
on 8 TRN2 NeuronCores.

Sharding: core c handles batch c//2 and heads (c%2)*6 .. (c%2)*6+6
(data parallel over batch x tensor parallel over head halves).
Each core computes a partial output (its 6 heads' contribution, [2048, 768]);
the host sums the two half-partials per batch and adds biases.

Device math (per core), all matmuls in fp16 with fp32 PSUM accumulation:
  QT[hd, q] = (Wq/8).T @ x.T + bq/8        (scale folded into Wq host-side)
  KT[hd, q] = Wk.T @ x.T + bk
  V[k, hd]  = x @ Wv                        ([V|1] augmented per head)
  ST[k, q]  = KT.T @ QT  (per head, causal strips k0..2048)
  PT = exp(ST) * tri-mask on diagonal blocks  (softmax w/o max-shift: scores
       are O(1) by construction, exp cannot overflow)
  z_aug = [V|1].T @ PT  accumulated over k-strips -> rows 0..63 = z, row 64 = sum
  zT = z_aug[0:64] / z_aug[64]              (recip + partition broadcast;
       zT stored pair-packed [128, q]: odd heads moved to rows 64..127 via DMA)
  out_partial[q, m] = sum_j ztp_j.T @ Wo_pair_j   (K=128 per pair)
Host: out[b] = partial[2b] + partial[2b+1] + b_O + sum_h b_V[h] @ W_O[h]
(b_V folds exactly through softmax rows summing to 1.)
"""
import contextlib
import numpy as np

import concourse.bass as bass
import concourse.mybir as mybir
import concourse.tile as tile
from concourse import bacc
from concourse.bass_utils import run_bass_kernel_spmd
from concourse.masks import make_upper_triangular, make_identity, make_lower_triangular

BATCH, SEQ, DM, NH, DH = 4, 2048, 768, 12, 64
H = 6                 # heads per core
HD = H * DH           # 384
MC = DM // 128        # 6 m-chunks
NKT = SEQ // 128      # 16 k-tiles
NQC = SEQ // 512      # 4 q-chunks
F16 = mybir.dt.float16
F32 = mybir.dt.float32

_BUILD_CACHE = {}


def build(reps: int = 1, upto: str = "all"):
    key = (reps, upto)
    if key in _BUILD_CACHE:
        return _BUILD_CACHE[key]
    nc = bacc.Bacc("TRN2", target_bir_lowering=False, debug=False)
    xt_d = nc.dram_tensor("xt", [DM, SEQ], F16, kind="ExternalInput")
    wq_d = nc.dram_tensor("wq", [DM, HD], F16, kind="ExternalInput")
    wk_d = nc.dram_tensor("wk", [DM, HD], F16, kind="ExternalInput")
    wv_d = nc.dram_tensor("wv", [DM, HD], F16, kind="ExternalInput")
    wo_d = nc.dram_tensor("wo", [3, 128, DM], F16, kind="ExternalInput")
    bq_d = nc.dram_tensor("bq", [HD], F32, kind="ExternalInput")
    bk_d = nc.dram_tensor("bk", [HD], F32, kind="ExternalInput")
    o_d = nc.dram_tensor("out", [SEQ, DM], F16, kind="ExternalOutput")

    with tile.TileContext(nc) as tc:
        def body(_iv=None):
            with contextlib.ExitStack() as ctx:
                consts = ctx.enter_context(tc.tile_pool(name="consts", bufs=1))
                persist = ctx.enter_context(tc.tile_pool(name="persist", bufs=1))

                # ---- load inputs (already fp16 from host) ----
                xt16 = []
                for c in range(MC):
                    xc = persist.tile([128, SEQ], F16, name=f"xt16_{c}")
                    nc.sync.dma_start(out=xc,
                                      in_=xt_d.ap()[c * 128:(c + 1) * 128, :])
                    xt16.append(xc)
                w16 = {}
                for name, d in [("wq", wq_d), ("wk", wk_d), ("wv", wv_d)]:
                    wt = persist.tile([128, MC, HD], F16, name=f"{name}16")
                    nc.sync.dma_start(
                        out=wt, in_=d.ap().rearrange("(c p) h -> p c h", p=128))
                    w16[name] = wt
                wo16 = []
                for j in range(3):
                    wot = persist.tile([128, DM], F16, name=f"wo16_{j}")
                    nc.sync.dma_start(out=wot, in_=wo_d.ap()[j])
                    wo16.append(wot)

                bq_s = consts.tile([128, HD // 128], F32)
                nc.sync.dma_start(
                    out=bq_s, in_=bq_d.ap().rearrange("(c p) -> p c", p=128))
                bk_s = consts.tile([128, HD // 128], F32)
                nc.sync.dma_start(
                    out=bk_s, in_=bk_d.ap().rearrange("(c p) -> p c", p=128))

                ident = consts.tile([128, 128], F16)
                make_identity(nc, ident)
                mneg = consts.tile([128, 128], F16)
                make_lower_triangular(nc, mneg, val=-30000.0, diag=False)

                if upto == "load":
                    return

                qt = [persist.tile([128, SEQ], F16, name=f"qt{j}")
                      for j in range(3)]
                kt_ = [persist.tile([128, SEQ], F16, name=f"kt{j}")
                      for j in range(3)]
                vt = [persist.tile([128, H, DH + 1], F16, name=f"v{i}")
                      for i in range(NKT)]
                # pair-packed zT: rows 0..63 head 2j, 64..127 head 2j+1
                ztp = [persist.tile([128, SEQ], F16, name=f"ztp{j}")
                       for j in range(3)]

                s2_ps = ctx.enter_context(
                    tc.tile_pool(name="s2_ps", bufs=2, space="PSUM"))
                s_ps = ctx.enter_context(
                    tc.tile_pool(name="s_ps", bufs=2, space="PSUM"))
                z_ps = ctx.enter_context(
                    tc.tile_pool(name="z_ps", bufs=2, space="PSUM"))
                m_ps = s_ps
                pt_pool = ctx.enter_context(tc.tile_pool(name="pt_pool", bufs=10))
                r_pool = ctx.enter_context(tc.tile_pool(name="r_pool", bufs=3))
                rb_pool = ctx.enter_context(tc.tile_pool(name="rb_pool", bufs=3))
                zo_pool = ctx.enter_context(tc.tile_pool(name="zo_pool", bufs=3))

                def proj_pair(j, with_v):
                    for dst, w, b_s in [(qt, "wq", bq_s), (kt_, "wk", bk_s)]:
                        for qc in range(NQC):
                            ps = m_ps.tile([128, 512], F32, name="ps", tag="s")
                            for c in range(MC):
                                nc.tensor.matmul(
                                    ps,
                                    w16[w][:, c, j * 128:(j + 1) * 128],
                                    xt16[c][:, qc * 512:(qc + 1) * 512],
                                    start=(c == 0), stop=(c == MC - 1))
                            nc.vector.tensor_scalar(
                                out=dst[j][:, qc * 512:(qc + 1) * 512],
                                in0=ps, scalar1=b_s[:, j:j + 1], scalar2=None,
                                op0=mybir.AluOpType.add)
                    if with_v:
                        for ktile in range(NKT):
                            ps = m_ps.tile([128, HD], F32, name="psv", tag="s")
                            for c in range(MC):
                                nc.tensor.matmul(
                                    ps,
                                    xt16[c][:, ktile * 128:(ktile + 1) * 128],
                                    w16["wv"][:, c, :],
                                    start=(c == 0), stop=(c == MC - 1))
                            nc.vector.tensor_copy(
                                vt[ktile][:, :, 0:DH],
                                ps.rearrange("p (h d) -> p h d", h=H))
                            nc.vector.memset(vt[ktile][:, :, DH:DH + 1], 1.0)

                def attn_head(h):
                    j, hp = h // 2, (h % 2) * 64
                    for qc in range(NQC):
                        qc0 = qc * 512
                        z_t = z_ps.tile([65, 512], F32, name="z_t", tag="z")
                        strips = []

                        def emit_pv(ktile, p_t, base, cs):
                            nc.tensor.matmul(
                                z_t[:, cs - qc0:512],
                                vt[ktile][:, h, :],
                                p_t[:, base + cs - qc0:base + 512],
                                start=(ktile == 0),
                                stop=(ktile == 4 * qc + 3))

                        def flush(n):
                            while len(strips) > n:
                                emit_pv(*strips.pop(0))

                        # paired full-width strips (non-diagonal)
                        for kp in range(2 * qc):
                            s_t = s2_ps.tile([128, 1024], F32, name="sp",
                                             tag="s2")
                            for idx in (0, 1):
                                ktile = 2 * kp + idx
                                k0 = ktile * 128
                                nc.tensor.matmul(
                                    s_t[:, idx * 512:idx * 512 + 512],
                                    kt_[j][hp:hp + 64, k0:k0 + 128],
                                    qt[j][hp:hp + 64, qc0:qc0 + 512],
                                    start=True, stop=True)
                            p_t = pt_pool.tile([128, 1024], F16, name="pp",
                                               tag="pt2", bufs=4)
                            nc.scalar.activation(
                                p_t, s_t, mybir.ActivationFunctionType.Exp)
                            strips.append((2 * kp, p_t, 0, qc0))
                            strips.append((2 * kp + 1, p_t, 512, qc0))
                            flush(2)
                        # diagonal strips
                        for ktile in range(4 * qc, 4 * qc + 4):
                            k0 = ktile * 128
                            cs = max(qc0, k0)
                            diag = cs == k0
                            s_t = s_ps.tile([128, 512], F32, name="s_t",
                                            tag="s")
                            nc.tensor.matmul(
                                s_t[:, cs - qc0:512],
                                kt_[j][hp:hp + 64, k0:k0 + 128],
                                qt[j][hp:hp + 64, cs:qc0 + 512],
                                start=True, stop=not diag)
                            if diag:
                                nc.tensor.matmul(
                                    s_t[:, k0 - qc0:k0 - qc0 + 128],
                                    ident, mneg, start=False, stop=True)
                            p_t = pt_pool.tile([128, 512], F16, name="p_t",
                                               tag="pt")
                            nc.scalar.activation(
                                p_t[:, cs - qc0:512], s_t[:, cs - qc0:512],
                                mybir.ActivationFunctionType.Exp)
                            strips.append((ktile, p_t, 0, cs))
                            flush(1)
                        flush(0)

                        # normalize -> pair-packed zT
                        r_t = r_pool.tile([65, 512], F32, name="r_t", tag="r")
                        nc.vector.reciprocal(out=r_t[64:65, :],
                                             in_=z_t[64:65, :])
                        r0_t = r_pool.tile([1, 512], F32, name="r0", tag="r0")
                        nc.sync.dma_start(out=r0_t, in_=r_t[64:65, :])
                        rb_t = rb_pool.tile([64, 512], F32, name="rb", tag="rb")
                        nc.gpsimd.partition_broadcast(rb_t, r0_t)
                        if hp == 0:
                            nc.vector.tensor_tensor(
                                out=ztp[j][0:64, qc0:qc0 + 512],
                                in0=z_t[0:64, :], in1=rb_t,
                                op=mybir.AluOpType.mult)
                        else:
                            zo_t = zo_pool.tile([64, 512], F16, name="zo",
                                                tag="zo")
                            nc.vector.tensor_tensor(
                                out=zo_t, in0=z_t[0:64, :], in1=rb_t,
                                op=mybir.AluOpType.mult)
                            nc.sync.dma_start(
                                out=ztp[j][64:128, qc0:qc0 + 512], in_=zo_t)

                proj_pair(0, with_v=True)
                if upto == "proj":
                    proj_pair(1, with_v=False)
                    proj_pair(2, with_v=False)
                    return
                for j in range(3):
                    if j:
                        proj_pair(j, with_v=False)
                    attn_head(2 * j)
                    attn_head(2 * j + 1)

                if upto == "attn":
                    return

                # ---- output projection (K=128 per pair) ----
                o_sb_pool = ctx.enter_context(tc.tile_pool(name="o_sb", bufs=3))
                for qtile in range(SEQ // 128):
                    o_s = o_sb_pool.tile([128, DM], F16, name="o_s", tag="os")
                    for n0, w in [(0, 512), (512, 256)]:
                        o_t = s2_ps.tile([128, 512], F32, name="o_t", tag="s2", bufs=2)
                        for j in range(3):
                            nc.tensor.matmul(
                                o_t[:, 0:w],
                                ztp[j][:, qtile * 128:(qtile + 1) * 128],
                                wo16[j][:, n0:n0 + w],
                                start=(j == 0), stop=(j == 2))
                        nc.vector.tensor_copy(o_s[:, n0:n0 + w], o_t[:, 0:w])
                    nc.sync.dma_start(
                        out=o_d.ap()[qtile * 128:(qtile + 1) * 128, :],
                        in_=o_s)

        if reps == 1:
            body()
        else:
            with tc.For_i(0, reps, 1) as _iv:
                body(_iv)

    nc.compile()
    _BUILD_CACHE[key] = nc
    return nc


def make_in_maps(normalized_resid_pre, W_Q, W_K, W_V, W_O, b_Q, b_K, b_V, b_O):
    scale = np.float32(1.0 / np.sqrt(DH))
    in_maps = []
    for core in range(8):
        b, h0 = core // 2, (core % 2) * H
        hs = slice(h0, h0 + H)
        in_maps.append({
            "xt": np.ascontiguousarray(
                normalized_resid_pre[b].T).astype(np.float16),
            "wq": (np.ascontiguousarray(
                W_Q[hs].transpose(1, 0, 2).reshape(DM, HD)) * scale).astype(
                np.float16),
            "wk": np.ascontiguousarray(
                W_K[hs].transpose(1, 0, 2).reshape(DM, HD)).astype(np.float16),
            "wv": np.ascontiguousarray(
                W_V[hs].transpose(1, 0, 2).reshape(DM, HD)).astype(np.float16),
            "wo": np.ascontiguousarray(
                W_O[hs].reshape(3, 128, DM)).astype(np.float16),
            "bq": (b_Q[hs].reshape(HD) * scale).astype(np.float32),
            "bk": b_K[hs].reshape(HD).astype(np.float32),
        })
    return in_maps


def assemble(results, b_V, b_O, W_O):
    bv_wo = np.einsum("hd,hdm->m", b_V.astype(np.float64),
                      W_O.astype(np.float64)).astype(np.float32)
    out = np.empty((BATCH, SEQ, DM), dtype=np.float32)
    for b in range(BATCH):
        out[b] = (results[2 * b]["out"].astype(np.float32)
                  + results[2 * b + 1]["out"].astype(np.float32)
                  + b_O + bv_wo)
    return out


def kernel(normalized_resid_pre, W_Q, W_K, W_V, W_O, b_Q, b_K, b_V, b_O):
    nc = build(reps=1)
    in_maps = make_in_maps(normalized_resid_pre, W_Q, W_K, W_V, W_O,
                           b_Q, b_K, b_V, b_O)
    last_err = None
    for _attempt in range(3):
        try:
            res = run_bass_kernel_spmd(nc, in_maps, core_ids=list(range(8)))
            return assemble(res.results, b_V, b_O, W_O)
        except Exception as e:  # transient NRT/axon hiccups observed
            last_err = e
    raise last_err
